# revision 6
# baseline (speedup 1.0000x reference)
"""Trainium2 Bass kernel for the MoE-routing module.

Computation (B=32768, D=1024, H=512, F=100, E=16, K=2):
    h   = relu(x @ W_shared + b_shared)                  [B, H]
    a   = relu(einsum('bh,ehf', h, W1) + b1)             [B, E, F]
    o   = einsum('bef,efo', a, W2) + b2                  [B, E, 1]
    out = mean over the K routed experts of o[b, send_to[idx[b]]]

Strategy: pure data-parallel over batch on 8 cores (4096 tokens each).
The routing is folded into a host-computed mask M[e, b] = (1/K) * count of
e among send_to[idx[b]], so the device computes
    out[b] = sum_e o[b, e] * M[e, b]
with three matmul stages, keeping features on SBUF partitions throughout:
  M1: hT[h, t]  = relu(W_shared.T @ xT)        lhsT=W_shared tiles
  M2: aT[f', t] = relu(W1cat.T @ hT)           f' = e*F + f  (E*F = 1600)
  M3: c[e, t]   = W2bd.T @ aT                  W2bd block-diagonal [1600, 16]
  sel: out[t]   = ones.T @ (c * mask)          1-partition result row
"""

import numpy as np

import concourse.mybir as mybir
from concourse import bacc
from concourse.bass_utils import run_bass_kernel_spmd
from concourse.tile import TileContext

B, D, H, F, E, TOPK = 32768, 1024, 512, 100, 16, 2
N_CORES = 8
BL = B // N_CORES          # tokens per core
CHUNK = 512                # tokens per device-side tile loop
N_CHUNKS = BL // CHUNK
EF = E * F                 # 1600
KT3 = (EF + 127) // 128    # 13 contraction tiles for M3
EF_PAD = KT3 * 128         # 1664

# Compute dtype for the matmul stages: "float32", "float32r", or "bfloat16"
import os
COMPUTE_DT = os.environ.get("KERNEL_DT", "float32")

_FP32 = mybir.dt.float32
_cache = {}


def _np_in_dtype():
    import ml_dtypes

    return ml_dtypes.bfloat16 if COMPUTE_DT == "bfloat16" else np.float32


def _build_nc():
    # CDT: dtype of matmul-feeding tensors (x, weights, hT, aT).
    # SDT: dtype of the tiny select stage (mask-mult + ones-matmul).
    CDT = getattr(mybir.dt, COMPUTE_DT)
    SDT = mybir.dt.bfloat16 if COMPUTE_DT == "bfloat16" else mybir.dt.float32

    def mm(ap):
        return ap
    nc = bacc.Bacc("TRN2", target_bir_lowering=False, num_devices=N_CORES)

    xT_d = nc.declare_dram_parameter("xT", [D, BL], CDT, isOutput=False)
    mask_d = nc.declare_dram_parameter("mask", [E, BL], _FP32, isOutput=False)
    wsh_d = nc.declare_dram_parameter("wsh", [D, H], CDT, isOutput=False)
    bsh_d = nc.declare_dram_parameter("bsh", [H], _FP32, isOutput=False)
    w1c_d = nc.declare_dram_parameter("w1c", [H, EF], CDT, isOutput=False)
    b1f_d = nc.declare_dram_parameter("b1f", [EF_PAD], _FP32, isOutput=False)
    w2bd_d = nc.declare_dram_parameter("w2bd", [EF_PAD, E], CDT, isOutput=False)
    b2_d = nc.declare_dram_parameter("b2", [E], _FP32, isOutput=False)
    out_d = nc.declare_dram_parameter("out", [BL], _FP32, isOutput=True)

    KD = D // 128   # 8 contraction tiles for M1
    MH = H // 128   # 4 output tiles for M1
    KH = H // 128   # 4 contraction tiles for M2
    relu = mybir.ActivationFunctionType.Relu

    with TileContext(nc) as tc:
        with (
            tc.tile_pool(name="weights", bufs=1) as wpool,
            tc.tile_pool(name="xin", bufs=2) as xpool,
            tc.tile_pool(name="mid", bufs=2) as midpool,
            tc.tile_pool(name="small", bufs=2) as spool,
            tc.tile_pool(name="ps_h", bufs=2, space="PSUM") as ps_h,
            tc.tile_pool(name="ps_a", bufs=2, space="PSUM") as ps_a,
            tc.tile_pool(name="ps_c", bufs=2, space="PSUM") as ps_c,
            tc.tile_pool(name="ps_o", bufs=2, space="PSUM") as ps_o,
        ):
            # ---- resident weights / biases ----
            # Order matters: wsh + chunk-0 x first so M1 starts ASAP; the
            # M2/M3 weights load while chunk-0 M1 runs. x/mask/out traffic
            # rides the GpSimd SWDGE queue, weights ride the Sync queue.
            xT_view = xT_d.rearrange("(o p) t -> p o t", p=128)
            wsh_sb = wpool.tile([128, KD, H], CDT)
            nc.sync.dma_start(wsh_sb[:], wsh_d.rearrange("(o p) h -> p o h", p=128))

            xts, masks = [], []
            for c in range(N_CHUNKS):
                t0 = c * CHUNK
                xt = xpool.tile([128, KD, CHUNK], CDT, tag="xt")
                nc.gpsimd.dma_start(xt[:], xT_view[:, :, t0 : t0 + CHUNK])
                mask_sb = spool.tile([E, CHUNK], _FP32, tag="mask")
                nc.gpsimd.dma_start(mask_sb[:], mask_d[:, t0 : t0 + CHUNK])
                xts.append(xt)
                masks.append(mask_sb)
                if c == 0:
                    w1c_sb = wpool.tile([128, KH, EF], CDT)
                    nc.sync.dma_start(w1c_sb[:], w1c_d.rearrange("(o p) f -> p o f", p=128))
                    w2bd_sb = wpool.tile([128, KT3, E], CDT)
                    nc.sync.dma_start(w2bd_sb[:], w2bd_d.rearrange("(o p) e -> p o e", p=128))
                    bsh_sb = wpool.tile([128, MH], _FP32)
                    nc.sync.dma_start(bsh_sb[:], bsh_d.rearrange("(o p) -> p o", p=128))
                    b1f_sb = wpool.tile([128, KT3], _FP32)
                    nc.sync.dma_start(b1f_sb[:], b1f_d.rearrange("(o p) -> p o", p=128))
                    b2_sb = wpool.tile([E, 1], _FP32)
                    nc.sync.dma_start(b2_sb[:], b2_d.rearrange("(e o) -> e o", o=1))
                    ones_sb = wpool.tile([E, 1], SDT)
                    nc.vector.memset(ones_sb[:], 1.0)

            for c in range(N_CHUNKS):
                t0 = c * CHUNK
                xt = xts[c]
                mask_sb = masks[c]

                # ---- M1: hT = relu(W_shared.T @ xT + b) ----
                hT = midpool.tile([128, MH, CHUNK], CDT, tag="hT")
                for m in range(MH):
                    ph = ps_h.tile([128, CHUNK], _FP32, tag="ps_h")
                    for k in range(KD):
                        nc.tensor.matmul(
                            ph[:],
                            lhsT=mm(wsh_sb[:, k, m * 128 : (m + 1) * 128]),
                            rhs=mm(xt[:, k, :]),
                            start=(k == 0),
                            stop=(k == KD - 1),
                        )
                    nc.scalar.activation(
                        hT[:, m, :], ph[:], relu, bias=bsh_sb[:, m : m + 1]
                    )

                # ---- M2: aT = relu(W1cat.T @ hT + b1) ----
                aT = midpool.tile([128, KT3, CHUNK], CDT, tag="aT")
                for m in range(KT3):
                    f0 = m * 128
                    fw = min(128, EF - f0)
                    pa = ps_a.tile([128, CHUNK], _FP32, tag="ps_a")
                    for k in range(KH):
                        nc.tensor.matmul(
                            pa[:fw],
                            lhsT=mm(w1c_sb[:, k, f0 : f0 + fw]),
                            rhs=mm(hT[:, k, :]),
                            start=(k == 0),
                            stop=(k == KH - 1),
                        )
                    nc.scalar.activation(
                        aT[:fw, m, :], pa[:fw], relu, bias=b1f_sb[:fw, m : m + 1]
                    )
                    if fw < 128:
                        nc.vector.memset(aT[fw:, m, :].bitcast(mybir.dt.float32), 0.0)

                # ---- M3: c = W2bd.T @ aT  (block-diag W2) ----
                pc = ps_c.tile([E, CHUNK], _FP32, tag="ps_c")
                for k in range(KT3):
                    nc.tensor.matmul(
                        pc[:],
                        lhsT=mm(w2bd_sb[:, k, :]),
                        rhs=mm(aT[:, k, :]),
                        start=(k == 0),
                        stop=(k == KT3 - 1),
                    )

                # ---- select: out = ones.T @ ((c + b2) * mask) ----
                msel = spool.tile([E, CHUNK], SDT, tag="msel")
                nc.vector.tensor_scalar_add(msel[:], pc[:], b2_sb[:])
                nc.vector.tensor_mul(msel[:], msel[:], mask_sb[:])
                po = ps_o.tile([1, CHUNK], _FP32, tag="ps_o")
                nc.tensor.matmul(po[:], lhsT=mm(ones_sb[:]), rhs=mm(msel[:]), start=True, stop=True)
                ot = spool.tile([1, CHUNK], _FP32, tag="ot")
                nc.vector.tensor_copy(ot[:], po[:])
                nc.gpsimd.dma_start(out_d[t0 : t0 + CHUNK].rearrange("(o t) -> o t", o=1), ot[:])

    nc.compile()
    return nc


def get_nc():
    key = COMPUTE_DT
    if key not in _cache:
        _cache[key] = _build_nc()
    return _cache[key]


def prepare_in_maps(inputs):
    """Host-side sharding + weight prep. Returns in_maps for 8 cores."""
    np_dt = _np_in_dtype()
    x = np.ascontiguousarray(np.asarray(inputs["x"], dtype=np.float32))
    idx = np.asarray(inputs["idx"]).astype(np.int64).reshape(B)
    W_shared = np.asarray(inputs["W_shared"], dtype=np.float32)
    b_shared = np.asarray(inputs["b_shared"], dtype=np.float32).reshape(H)
    W1 = np.asarray(inputs["W1"], dtype=np.float32)
    b1 = np.asarray(inputs["b1"], dtype=np.float32)
    W2 = np.asarray(inputs["W2"], dtype=np.float32)
    b2 = np.asarray(inputs["b2"], dtype=np.float32).reshape(E)
    send_to = np.asarray(inputs["send_to"]).astype(np.int64)

    # routing mask: mask[e, b] = (1/TOPK) * |{k : send_to[idx[b], k] == e}|
    routes = send_to[idx]  # [B, K]
    mask = np.zeros((E, B), dtype=np.float32)
    for k in range(routes.shape[1]):
        np.add.at(mask, (routes[:, k], np.arange(B)), 1.0 / routes.shape[1])

    w1c = np.ascontiguousarray(W1.transpose(1, 0, 2).reshape(H, EF)).astype(np_dt)
    b1f = np.zeros(EF_PAD, dtype=np.float32)
    b1f[:EF] = b1.reshape(EF)
    w2bd = np.zeros((EF_PAD, E), dtype=np.float32)
    for e in range(E):
        w2bd[e * F : (e + 1) * F, e] = W2[e, :, 0]
    w2bd = w2bd.astype(np_dt)
    wsh = np.ascontiguousarray(W_shared).astype(np_dt)

    in_maps = []
    for c in range(N_CORES):
        sl = slice(c * BL, (c + 1) * BL)
        in_maps.append(
            {
                "xT": np.ascontiguousarray(x[sl].T).astype(np_dt),
                "mask": np.ascontiguousarray(mask[:, sl]),
                "wsh": wsh,
                "bsh": b_shared,
                "w1c": w1c,
                "b1f": b1f,
                "w2bd": w2bd,
                "b2": b2,
            }
        )
    return in_maps


def kernel(**inputs) -> np.ndarray:
    nc = get_nc()
    in_maps = prepare_in_maps(inputs)
    res = run_bass_kernel_spmd(nc, in_maps, list(range(N_CORES)))
    out = np.concatenate([res.results[c]["out"] for c in range(N_CORES)])
    return out.reshape(B, 1).astype(np.float32)


# revision 9
# speedup vs baseline: 1.0132x; 1.0132x over previous
"""Trainium2 Bass kernel for the MoE-routing module.

Computation (B=32768, D=1024, H=512, F=100, E=16, K=2):
    h   = relu(x @ W_shared + b_shared)                  [B, H]
    a   = relu(einsum('bh,ehf', h, W1) + b1)             [B, E, F]
    o   = einsum('bef,efo', a, W2) + b2                  [B, E, 1]
    out = mean over the K routed experts of o[b, send_to[idx[b]]]

Strategy: pure data-parallel over batch on 8 cores (4096 tokens each).
The routing is folded into a host-computed mask M[e, b] = (1/K) * count of
e among send_to[idx[b]], so the device computes
    out[b] = sum_e o[b, e] * M[e, b]
with three matmul stages, keeping features on SBUF partitions throughout:
  M1: hT[h, t]  = relu(W_shared.T @ xT)        lhsT=W_shared tiles
  M2: aT[f', t] = relu(W1cat.T @ hT)           f' = e*F + f  (E*F = 1600)
  M3: c[e, t]   = W2bd.T @ aT                  W2bd block-diagonal [1600, 16]
  sel: out[t]   = ones.T @ (c * mask)          1-partition result row
"""

import numpy as np

import concourse.mybir as mybir
from concourse import bacc
from concourse.bass_utils import run_bass_kernel_spmd
from concourse.tile import TileContext

B, D, H, F, E, TOPK = 32768, 1024, 512, 100, 16, 2
N_CORES = 8
BL = B // N_CORES          # tokens per core
CHUNK = 512                # tokens per device-side tile loop
N_CHUNKS = BL // CHUNK
EF = E * F                 # 1600
KT3 = (EF + 127) // 128    # 13 contraction tiles for M3
EF_PAD = KT3 * 128         # 1664

# Compute dtype for the matmul stages: "float32", "float32r", or "bfloat16"
import os
COMPUTE_DT = os.environ.get("KERNEL_DT", "float32")

_FP32 = mybir.dt.float32
_cache = {}


def _np_in_dtype():
    import ml_dtypes

    return ml_dtypes.bfloat16 if COMPUTE_DT == "bfloat16" else np.float32


def _build_nc():
    # CDT: dtype of matmul-feeding tensors (x, weights, hT, aT).
    # SDT: dtype of the tiny select stage (mask-mult + ones-matmul).
    CDT = getattr(mybir.dt, COMPUTE_DT)
    SDT = mybir.dt.bfloat16 if COMPUTE_DT == "bfloat16" else mybir.dt.float32

    def mm(ap):
        return ap
    nc = bacc.Bacc("TRN2", target_bir_lowering=False, num_devices=N_CORES)

    xT_d = nc.declare_dram_parameter("xT", [D, BL], CDT, isOutput=False)
    mask_d = nc.declare_dram_parameter("mask", [E, BL], _FP32, isOutput=False)
    wsh_d = nc.declare_dram_parameter("wsh", [D, H], CDT, isOutput=False)
    bsh_d = nc.declare_dram_parameter("biases", [128, (H // 128) + KT3 + 1], _FP32, isOutput=False)
    w1c_d = nc.declare_dram_parameter("w1c", [H, EF], CDT, isOutput=False)
    w2bd_d = nc.declare_dram_parameter("w2bd", [128, KT3 * E], CDT, isOutput=False)
    out_d = nc.declare_dram_parameter("out", [BL], _FP32, isOutput=True)

    KD = D // 128   # 8 contraction tiles for M1
    MH = H // 128   # 4 output tiles for M1
    KH = H // 128   # 4 contraction tiles for M2
    relu = mybir.ActivationFunctionType.Relu

    with TileContext(nc) as tc:
        with (
            tc.tile_pool(name="weights", bufs=1) as wpool,
            tc.tile_pool(name="xin", bufs=2) as xpool,
            tc.tile_pool(name="mid", bufs=2) as midpool,
            tc.tile_pool(name="small", bufs=2) as spool,
            tc.tile_pool(name="ps_h", bufs=2, space="PSUM") as ps_h,
            tc.tile_pool(name="ps_a", bufs=2, space="PSUM") as ps_a,
            tc.tile_pool(name="ps_c", bufs=2, space="PSUM") as ps_c,
            tc.tile_pool(name="ps_o", bufs=2, space="PSUM") as ps_o,
        ):
            # ---- resident weights / biases ----
            # Order matters: wsh + chunk-0 x first so M1 starts ASAP; the
            # M2/M3 weights load while chunk-0 M1 runs. x/mask/out traffic
            # rides the GpSimd SWDGE queue, weights ride the Sync queue.
            xT_view = xT_d.rearrange("(o p) t -> p o t", p=128)
            wsh_sb = wpool.tile([128, KD, H], CDT)
            with tc.high_priority():
                nc.sync.dma_start(wsh_sb[:], wsh_d.rearrange("(o p) h -> p o h", p=128))

            xts, masks = [], []
            for c in range(N_CHUNKS):
                t0 = c * CHUNK
                xt = xpool.tile([128, KD, CHUNK], CDT, tag="xt")
                nc.scalar.dma_start(xt[:], xT_view[:, :, t0 : t0 + CHUNK])
                mask_sb = spool.tile([E, CHUNK], _FP32, tag="mask")
                nc.scalar.dma_start(mask_sb[:], mask_d[:, t0 : t0 + CHUNK])
                xts.append(xt)
                masks.append(mask_sb)
                if c == 0:
                    w1c_sb = wpool.tile([128, KH, EF], CDT)
                    nc.sync.dma_start(w1c_sb[:], w1c_d.rearrange("(o p) f -> p o f", p=128))
                    w2bd_sb = wpool.tile([128, KT3 * E], CDT)
                    nc.sync.dma_start(w2bd_sb[:], w2bd_d[:])
                    bias_sb = wpool.tile([128, MH + KT3 + 1], _FP32)
                    nc.sync.dma_start(bias_sb[:], bsh_d[:])
                    bsh_sb = bias_sb
                    b1f_sb = bias_sb[:, MH : MH + KT3]
                    b2_sb = bias_sb[:E, MH + KT3 : MH + KT3 + 1]
                    ones_sb = wpool.tile([E, 1], SDT)
                    nc.vector.memset(ones_sb[:], 1.0)

            for c in range(N_CHUNKS):
                t0 = c * CHUNK
                xt = xts[c]
                mask_sb = masks[c]

                # ---- M1: hT = relu(W_shared.T @ xT + b) ----
                hT = midpool.tile([128, MH, CHUNK], CDT, tag="hT")
                for m in range(MH):
                    ph = ps_h.tile([128, CHUNK], _FP32, tag="ps_h")
                    for k in range(KD):
                        nc.tensor.matmul(
                            ph[:],
                            lhsT=mm(wsh_sb[:, k, m * 128 : (m + 1) * 128]),
                            rhs=mm(xt[:, k, :]),
                            start=(k == 0),
                            stop=(k == KD - 1),
                        )
                    nc.scalar.activation(
                        hT[:, m, :], ph[:], relu, bias=bsh_sb[:, m : m + 1]
                    )

                # ---- M2: aT = relu(W1cat.T @ hT + b1) ----
                aT = midpool.tile([128, KT3, CHUNK], CDT, tag="aT")
                for m in range(KT3):
                    f0 = m * 128
                    fw = min(128, EF - f0)
                    pa = ps_a.tile([128, CHUNK], _FP32, tag="ps_a")
                    for k in range(KH):
                        nc.tensor.matmul(
                            pa[:fw],
                            lhsT=mm(w1c_sb[:, k, f0 : f0 + fw]),
                            rhs=mm(hT[:, k, :]),
                            start=(k == 0),
                            stop=(k == KH - 1),
                        )
                    nc.scalar.activation(
                        aT[:fw, m, :], pa[:fw], relu, bias=b1f_sb[:fw, m : m + 1]
                    )
                    if fw < 128:
                        nc.vector.memset(aT[fw:, m, :].bitcast(mybir.dt.float32), 0.0)

                # ---- M3: c = W2bd.T @ aT  (block-diag W2) ----
                pc = ps_c.tile([E, CHUNK], _FP32, tag="ps_c")
                for k in range(KT3):
                    nc.tensor.matmul(
                        pc[:],
                        lhsT=mm(w2bd_sb[:, k * E : (k + 1) * E]),
                        rhs=mm(aT[:, k, :]),
                        start=(k == 0),
                        stop=(k == KT3 - 1),
                    )

                # ---- select: out = ones.T @ ((c + b2) * mask) ----
                msel = spool.tile([E, CHUNK], SDT, tag="msel")
                nc.vector.tensor_scalar_add(msel[:], pc[:], b2_sb)
                nc.vector.tensor_mul(msel[:], msel[:], mask_sb[:])
                po = ps_o.tile([1, CHUNK], _FP32, tag="ps_o")
                nc.tensor.matmul(po[:], lhsT=mm(ones_sb[:]), rhs=mm(msel[:]), start=True, stop=True)
                ot = spool.tile([1, CHUNK], _FP32, tag="ot")
                nc.vector.tensor_copy(ot[:], po[:])
                nc.gpsimd.dma_start(out_d[t0 : t0 + CHUNK].rearrange("(o t) -> o t", o=1), ot[:])

    nc.compile()
    return nc


def get_nc():
    key = COMPUTE_DT
    if key not in _cache:
        _cache[key] = _build_nc()
    return _cache[key]


def prepare_in_maps(inputs):
    """Host-side sharding + weight prep. Returns in_maps for 8 cores."""
    np_dt = _np_in_dtype()
    x = np.ascontiguousarray(np.asarray(inputs["x"], dtype=np.float32))
    idx = np.asarray(inputs["idx"]).astype(np.int64).reshape(B)
    W_shared = np.asarray(inputs["W_shared"], dtype=np.float32)
    b_shared = np.asarray(inputs["b_shared"], dtype=np.float32).reshape(H)
    W1 = np.asarray(inputs["W1"], dtype=np.float32)
    b1 = np.asarray(inputs["b1"], dtype=np.float32)
    W2 = np.asarray(inputs["W2"], dtype=np.float32)
    b2 = np.asarray(inputs["b2"], dtype=np.float32).reshape(E)
    send_to = np.asarray(inputs["send_to"]).astype(np.int64)

    # routing mask: mask[e, b] = (1/TOPK) * |{k : send_to[idx[b], k] == e}|
    routes = send_to[idx]  # [B, K]
    mask = np.zeros((E, B), dtype=np.float32)
    for k in range(routes.shape[1]):
        np.add.at(mask, (routes[:, k], np.arange(B)), 1.0 / routes.shape[1])

    w1c = np.ascontiguousarray(W1.transpose(1, 0, 2).reshape(H, EF)).astype(np_dt)
    MH = H // 128
    biases = np.zeros((128, MH + KT3 + 1), dtype=np.float32)
    biases[:, :MH] = b_shared.reshape(MH, 128).T
    b1f = np.zeros(EF_PAD, dtype=np.float32)
    b1f[:EF] = b1.reshape(EF)
    biases[:, MH : MH + KT3] = b1f.reshape(KT3, 128).T
    biases[:E, MH + KT3] = b2
    w2bd_full = np.zeros((EF_PAD, E), dtype=np.float32)
    for e in range(E):
        w2bd_full[e * F : (e + 1) * F, e] = W2[e, :, 0]
    # pack [KT3*128, E] -> [128, KT3*E] so the DMA rows are contiguous
    w2bd = np.ascontiguousarray(
        w2bd_full.reshape(KT3, 128, E).transpose(1, 0, 2).reshape(128, KT3 * E)
    ).astype(np_dt)
    wsh = np.ascontiguousarray(W_shared).astype(np_dt)

    in_maps = []
    for c in range(N_CORES):
        sl = slice(c * BL, (c + 1) * BL)
        in_maps.append(
            {
                "xT": np.ascontiguousarray(x[sl].T).astype(np_dt),
                "mask": np.ascontiguousarray(mask[:, sl]),
                "wsh": wsh,
                "biases": biases,
                "w1c": w1c,
                "w2bd": w2bd,
            }
        )
    return in_maps


def kernel(**inputs) -> np.ndarray:
    nc = get_nc()
    in_maps = prepare_in_maps(inputs)
    res = run_bass_kernel_spmd(nc, in_maps, list(range(N_CORES)))
    out = np.concatenate([res.results[c]["out"] for c in range(N_CORES)])
    return out.reshape(B, 1).astype(np.float32)


# revision 11
# speedup vs baseline: 1.6485x; 1.6271x over previous
"""Trainium2 Bass kernel for the MoE-routing module.

Computation (B=32768, D=1024, H=512, F=100, E=16, K=2):
    h   = relu(x @ W_shared + b_shared)                  [B, H]
    a   = relu(einsum('bh,ehf', h, W1) + b1)             [B, E, F]
    o   = einsum('bef,efo', a, W2) + b2                  [B, E, 1]
    out = mean over the K routed experts of o[b, send_to[idx[b]]]

Strategy: host sorts tokens by head id and shards the sorted batch over the
8 cores (4096 tokens each, perfectly balanced).  A sorted 4096-token window
only routes to a handful of consecutive experts, so each core gets just the
expert slices it needs (EC slots, adaptively >= actual need; EC=16 degrades
to the dense all-expert kernel).  Routing is folded into a host-computed
per-slot mask M[j, b], so the device computes
    out[b] = sum_j o_local[b, j] * M[j, b]
with three matmul stages, features on SBUF partitions throughout:
  M1: hT[h, t]  = relu(W_shared.T @ xT)         lhsT = W_shared tiles
  M2: aT[f', t] = relu(W1sel.T @ hT)            f' = j*F + f  (EC*F wide)
  M3: c[j, t]   = W2sel.T @ aT                  W2sel block-diagonal
  sel: out[t]   = ones.T @ (c * mask)           1-partition result row
All matmuls run as float32r (full-rate fp32 mode, ~1e-4 rel err).
"""

import os

import numpy as np

import concourse.mybir as mybir
from concourse import bacc
from concourse.bass_utils import run_bass_kernel_spmd
from concourse.tile import TileContext

B, D, H, F, E, TOPK = 32768, 1024, 512, 100, 16, 2
N_CORES = 8
BL = B // N_CORES          # tokens per core
CHUNK = 512                # tokens per device-side tile loop
N_CHUNKS = BL // CHUNK
MH = H // 128              # M1 output tiles
KD = D // 128              # M1 contraction tiles
KH = H // 128              # M2 contraction tiles
EC_MIN = 5                 # minimum expert slots per core

# Compute dtype for the matmul stages: "float32", "float32r", or "bfloat16"
COMPUTE_DT = os.environ.get("KERNEL_DT", "float32r")

_FP32 = mybir.dt.float32
_cache = {}


def _np_in_dtype():
    import ml_dtypes

    return ml_dtypes.bfloat16 if COMPUTE_DT == "bfloat16" else np.float32


def _build_nc(ec):
    """Build the SPMD program for EC expert slots per core."""
    CDT = getattr(mybir.dt, COMPUTE_DT)
    SDT = mybir.dt.bfloat16 if COMPUTE_DT == "bfloat16" else mybir.dt.float32
    EF = ec * F                    # local expert-concat width
    KT3 = (EF + 127) // 128        # M2 output tiles / M3 contraction tiles
    EF_PAD = KT3 * 128             # w1sel zero-padded so all tiles are full
    NB = MH + KT3 + 1              # packed bias columns

    nc = bacc.Bacc("TRN2", target_bir_lowering=False, num_devices=N_CORES)

    xT_d = nc.declare_dram_parameter("xT", [N_CHUNKS, D, CHUNK], CDT, isOutput=False)
    mask_d = nc.declare_dram_parameter("mask", [ec, BL], _FP32, isOutput=False)
    wsh_d = nc.declare_dram_parameter("wsh", [D, H], CDT, isOutput=False)
    w1c_d = nc.declare_dram_parameter("w1c", [H, EF_PAD], CDT, isOutput=False)
    w2bd_d = nc.declare_dram_parameter("w2bd", [128, KT3 * ec], CDT, isOutput=False)
    bias_d = nc.declare_dram_parameter("biases", [128, NB], _FP32, isOutput=False)
    out_d = nc.declare_dram_parameter("out", [BL], _FP32, isOutput=True)

    relu = mybir.ActivationFunctionType.Relu

    with TileContext(nc) as tc:
        with (
            tc.tile_pool(name="weights", bufs=1) as wpool,
            tc.tile_pool(name="xin", bufs=2) as xpool,
            tc.tile_pool(name="mid", bufs=2) as midpool,
            tc.tile_pool(name="small", bufs=2) as spool,
            tc.tile_pool(name="ps_h", bufs=2, space="PSUM") as ps_h,
            tc.tile_pool(name="ps_a", bufs=2, space="PSUM") as ps_a,
            tc.tile_pool(name="ps_c", bufs=2, space="PSUM") as ps_c,
            tc.tile_pool(name="ps_o", bufs=2, space="PSUM") as ps_o,
        ):
            # ---- weights: wsh first (M1-critical), M2/M3 weights after the
            # first x chunk is queued.  Weights ride the Sync HWDGE queue,
            # x/mask ride the Activation HWDGE queue, out rides GpSimd.
            wsh_sb = wpool.tile([128, KD, H], CDT)
            with tc.high_priority():
                nc.sync.dma_start(wsh_sb[:], wsh_d.rearrange("(o p) h -> p o h", p=128))

            xts, masks = [], []
            for c in range(N_CHUNKS):
                xt = xpool.tile([128, KD, CHUNK], CDT, tag="xt")
                nc.scalar.dma_start(xt[:], xT_d[c].rearrange("(o p) t -> p o t", p=128))
                mask_sb = spool.tile([ec, CHUNK], _FP32, tag="mask")
                nc.scalar.dma_start(mask_sb[:], mask_d[:, c * CHUNK : (c + 1) * CHUNK])
                xts.append(xt)
                masks.append(mask_sb)
                if c == 0:
                    w1c_sb = wpool.tile([128, KH, EF_PAD], CDT)
                    nc.sync.dma_start(w1c_sb[:], w1c_d.rearrange("(o p) f -> p o f", p=128))
                    w2bd_sb = wpool.tile([128, KT3 * ec], CDT)
                    nc.sync.dma_start(w2bd_sb[:], w2bd_d[:])
                    bias_sb = wpool.tile([128, NB], _FP32)
                    nc.sync.dma_start(bias_sb[:], bias_d[:])
                    b2_sb = bias_sb[:ec, MH + KT3 : MH + KT3 + 1]
                    ones_sb = wpool.tile([ec, 1], SDT)
                    nc.vector.memset(ones_sb[:], 1.0)

            for c in range(N_CHUNKS):
                t0 = c * CHUNK
                xt = xts[c]
                mask_sb = masks[c]

                # ---- M1: hT = relu(W_shared.T @ xT + b) ----
                hT = midpool.tile([128, MH, CHUNK], CDT, tag="hT")
                for m in range(MH):
                    ph = ps_h.tile([128, CHUNK], _FP32, tag="ps_h")
                    for k in range(KD):
                        nc.tensor.matmul(
                            ph[:],
                            lhsT=wsh_sb[:, k, m * 128 : (m + 1) * 128],
                            rhs=xt[:, k, :],
                            start=(k == 0),
                            stop=(k == KD - 1),
                        )
                    nc.scalar.activation(
                        hT[:, m, :], ph[:], relu, bias=bias_sb[:, m : m + 1]
                    )

                # ---- M2: aT = relu(W1sel.T @ hT + b1) ----
                aT = midpool.tile([128, KT3, CHUNK], CDT, tag="aT")
                for m in range(KT3):
                    f0 = m * 128
                    pa = ps_a.tile([128, CHUNK], _FP32, tag="ps_a")
                    for k in range(KH):
                        nc.tensor.matmul(
                            pa[:],
                            lhsT=w1c_sb[:, k, f0 : f0 + 128],
                            rhs=hT[:, k, :],
                            start=(k == 0),
                            stop=(k == KH - 1),
                        )
                    nc.scalar.activation(
                        aT[:, m, :], pa[:], relu,
                        bias=bias_sb[:, MH + m : MH + m + 1],
                    )

                # ---- M3: c = W2sel.T @ aT  (block-diag W2) ----
                pc = ps_c.tile([ec, CHUNK], _FP32, tag="ps_c")
                for k in range(KT3):
                    nc.tensor.matmul(
                        pc[:],
                        lhsT=w2bd_sb[:, k * ec : (k + 1) * ec],
                        rhs=aT[:, k, :],
                        start=(k == 0),
                        stop=(k == KT3 - 1),
                    )

                # ---- select: out = ones.T @ ((c + b2) * mask) ----
                msel = spool.tile([ec, CHUNK], SDT, tag="msel")
                nc.vector.tensor_scalar_add(msel[:], pc[:], b2_sb)
                nc.vector.tensor_mul(msel[:], msel[:], mask_sb[:])
                po = ps_o.tile([1, CHUNK], _FP32, tag="ps_o")
                nc.tensor.matmul(po[:], lhsT=ones_sb[:], rhs=msel[:], start=True, stop=True)
                ot = spool.tile([1, CHUNK], _FP32, tag="ot")
                nc.vector.tensor_copy(ot[:], po[:])
                nc.gpsimd.dma_start(out_d[t0 : t0 + CHUNK].rearrange("(o t) -> o t", o=1), ot[:])

    nc.compile()
    return nc


def get_nc(ec):
    key = (COMPUTE_DT, ec)
    if key not in _cache:
        _cache[key] = _build_nc(ec)
    return _cache[key]


def prepare(inputs):
    """Host-side routing/sorting/sharding. Returns (ec, in_maps, perm)."""
    np_dt = _np_in_dtype()
    x = np.asarray(inputs["x"], dtype=np.float32)
    idx = np.asarray(inputs["idx"]).astype(np.int64).reshape(B)
    W_shared = np.asarray(inputs["W_shared"], dtype=np.float32)
    b_shared = np.asarray(inputs["b_shared"], dtype=np.float32).reshape(H)
    W1 = np.asarray(inputs["W1"], dtype=np.float32)
    b1 = np.asarray(inputs["b1"], dtype=np.float32).reshape(E, F)
    W2 = np.asarray(inputs["W2"], dtype=np.float32).reshape(E, F)
    b2 = np.asarray(inputs["b2"], dtype=np.float32).reshape(E)
    send_to = np.asarray(inputs["send_to"]).astype(np.int64)

    perm = np.argsort(idx, kind="stable")
    idx_s = idx[perm]
    routes_s = send_to[idx_s]                      # [B, K] sorted routes
    x_s = x[perm]                                  # [B, D]

    # per-core expert lists
    expert_lists = []
    for c in range(N_CORES):
        sl = slice(c * BL, (c + 1) * BL)
        expert_lists.append(np.unique(routes_s[sl]))
    ec = max(EC_MIN, max(len(el) for el in expert_lists))
    ec = min(ec, E)

    wsh = np.ascontiguousarray(W_shared).astype(np_dt)
    EF = ec * F
    KT3 = (EF + 127) // 128
    EF_PAD = KT3 * 128
    NB = MH + KT3 + 1

    in_maps = []
    for c in range(N_CORES):
        sl = slice(c * BL, (c + 1) * BL)
        el = expert_lists[c]
        # local slot tables (pad slots use sentinel -1: zero weights, no mask)
        slots = np.full(ec, -1, dtype=np.int64)
        slots[: len(el)] = el

        # mask[j, b] = (1/K) * count of slots[j] among routes of token b
        r = routes_s[sl]                            # [BL, K]
        mask = np.zeros((ec, BL), dtype=np.float32)
        for k in range(r.shape[1]):
            hit = slots[:, None] == r[None, :, k]   # [ec, BL]
            mask += hit.astype(np.float32) / r.shape[1]

        w1sel = np.zeros((H, EF_PAD), dtype=np.float32)
        b1sel = np.zeros(EF_PAD, dtype=np.float32)
        w2full = np.zeros((EF_PAD, ec), dtype=np.float32)
        for j, e in enumerate(slots):
            if e < 0:
                continue
            w1sel[:, j * F : (j + 1) * F] = W1[e]
            b1sel[j * F : (j + 1) * F] = b1[e]
            w2full[j * F : (j + 1) * F, j] = W2[e]
        w2bd = np.ascontiguousarray(
            w2full.reshape(KT3, 128, ec).transpose(1, 0, 2).reshape(128, KT3 * ec)
        ).astype(np_dt)

        biases = np.zeros((128, NB), dtype=np.float32)
        biases[:, :MH] = b_shared.reshape(MH, 128).T
        biases[:, MH : MH + KT3] = b1sel.reshape(KT3, 128).T
        biases[:ec, MH + KT3] = b2[np.maximum(slots, 0)] * (slots >= 0)

        xT = np.ascontiguousarray(
            x_s[sl].reshape(N_CHUNKS, CHUNK, D).transpose(0, 2, 1)
        ).astype(np_dt)

        in_maps.append(
            {
                "xT": xT,
                "mask": mask,
                "wsh": wsh,
                "w1c": w1sel.astype(np_dt),
                "w2bd": w2bd,
                "biases": biases,
            }
        )
    return ec, in_maps, perm


def kernel(**inputs) -> np.ndarray:
    ec, in_maps, perm = prepare(inputs)
    nc = get_nc(ec)
    res = run_bass_kernel_spmd(nc, in_maps, list(range(N_CORES)))
    out_sorted = np.concatenate([res.results[c]["out"] for c in range(N_CORES)])
    out = np.empty(B, dtype=np.float32)
    out[perm] = out_sorted
    return out.reshape(B, 1)


# revision 13
# speedup vs baseline: 1.7356x; 1.0528x over previous
"""Trainium2 Bass kernel for the MoE-routing module.

Computation (B=32768, D=1024, H=512, F=100, E=16, K=2):
    h   = relu(x @ W_shared + b_shared)                  [B, H]
    a   = relu(einsum('bh,ehf', h, W1) + b1)             [B, E, F]
    o   = einsum('bef,efo', a, W2) + b2                  [B, E, 1]
    out = mean over the K routed experts of o[b, send_to[idx[b]]]

Strategy: host sorts tokens by head id and shards the sorted batch over the
8 cores (4096 tokens each, perfectly balanced).  A sorted 4096-token window
only routes to a handful of consecutive experts, so each core gets just the
expert slices it needs (EC slots, adaptively >= actual need; EC=16 degrades
to the dense all-expert kernel).  Routing is folded into a host-computed
per-slot mask M[j, b], so the device computes
    out[b] = sum_j o_local[b, j] * M[j, b]
with three matmul stages, features on SBUF partitions throughout:
  M1: hT[h, t]  = relu(W_shared.T @ xT)         lhsT = W_shared tiles
  M2: aT[f', t] = relu(W1sel.T @ hT)            f' = j*F + f  (EC*F wide)
  M3: c[j, t]   = W2sel.T @ aT                  W2sel block-diagonal
  sel: out[t]   = ones.T @ (c * mask)           1-partition result row
All matmuls run as float32r (full-rate fp32 mode, ~1e-4 rel err).
"""

import os

import numpy as np

import concourse.mybir as mybir
from concourse import bacc
from concourse.bass_utils import run_bass_kernel_spmd
from concourse.tile import TileContext

B, D, H, F, E, TOPK = 32768, 1024, 512, 100, 16, 2
N_CORES = 8
BL = B // N_CORES          # tokens per core
CHUNK = 512                # tokens per device-side tile loop
N_CHUNKS = BL // CHUNK
MH = H // 128              # M1 output tiles
KD = D // 128              # M1 contraction tiles
KH = H // 128              # M2 contraction tiles
EC_MIN = 5                 # minimum expert slots per core

# Compute dtype for the matmul stages: "float32", "float32r", or "bfloat16"
COMPUTE_DT = os.environ.get("KERNEL_DT", "float32r")

_FP32 = mybir.dt.float32
_cache = {}


def _np_in_dtype():
    import ml_dtypes

    return ml_dtypes.bfloat16 if COMPUTE_DT == "bfloat16" else np.float32


def _build_nc(ec):
    """Build the SPMD program for EC expert slots per core."""
    CDT = getattr(mybir.dt, COMPUTE_DT)
    SDT = mybir.dt.bfloat16 if COMPUTE_DT == "bfloat16" else mybir.dt.float32
    EF = ec * F                    # local expert-concat width
    KT3 = (EF + 127) // 128        # M2 output tiles / M3 contraction tiles
    EF_PAD = KT3 * 128             # w1sel zero-padded so all tiles are full
    NB = MH + KT3 + 1              # packed bias columns

    nc = bacc.Bacc("TRN2", target_bir_lowering=False, num_devices=N_CORES)

    xT_d = nc.declare_dram_parameter("xT", [N_CHUNKS, D, CHUNK], CDT, isOutput=False)
    mask_d = nc.declare_dram_parameter("mask", [ec, BL], _FP32, isOutput=False)
    wsh_d = nc.declare_dram_parameter("wsh", [D, H], CDT, isOutput=False)
    w1c_d = nc.declare_dram_parameter("w1c", [H, EF_PAD], CDT, isOutput=False)
    w2bd_d = nc.declare_dram_parameter("w2bd", [128, KT3 * ec], CDT, isOutput=False)
    bias_d = nc.declare_dram_parameter("biases", [128, NB], _FP32, isOutput=False)
    out_d = nc.declare_dram_parameter("out", [BL], _FP32, isOutput=True)

    relu = mybir.ActivationFunctionType.Relu

    with TileContext(nc) as tc:
        with (
            tc.tile_pool(name="weights", bufs=1) as wpool,
            tc.tile_pool(name="xin", bufs=2) as xpool,
            tc.tile_pool(name="mid", bufs=2) as midpool,
            tc.tile_pool(name="small", bufs=2) as spool,
            tc.tile_pool(name="ps_h", bufs=4, space="PSUM") as ps_h,
            tc.tile_pool(name="ps_a", bufs=2, space="PSUM") as ps_a,
            tc.tile_pool(name="ps_c", bufs=1, space="PSUM") as ps_c,
            tc.tile_pool(name="ps_o", bufs=1, space="PSUM") as ps_o,
        ):
            # ---- startup-critical loads: wsh + chunk-0 x, split per k-tile
            # and interleaved across both HWDGE queues (Sync + Activation)
            # so M1 of chunk 0 can start after the first ~512KB lands.
            wsh_view = wsh_d.rearrange("(o p) h -> p o h", p=128)
            wsh_sb = wpool.tile([128, KD, H], CDT)
            xt0 = xpool.tile([128, KD, CHUNK], CDT, tag="xt")
            xt0_view = xT_d[0].rearrange("(o p) t -> p o t", p=128)
            with tc.high_priority():
                for k in range(KD):
                    qa = nc.sync if k % 2 == 0 else nc.scalar
                    qb = nc.scalar if k % 2 == 0 else nc.sync
                    qa.dma_start(wsh_sb[:, k], wsh_view[:, k])
                    qb.dma_start(xt0[:, k], xt0_view[:, k])

            xts, masks = [xt0], []
            for c in range(N_CHUNKS):
                if c > 0:
                    xt = xpool.tile([128, KD, CHUNK], CDT, tag="xt")
                    xv = xT_d[c].rearrange("(o p) t -> p o t", p=128)
                    nc.scalar.dma_start(xt[:, : KD // 2], xv[:, : KD // 2])
                    nc.sync.dma_start(xt[:, KD // 2 :], xv[:, KD // 2 :])
                    xts.append(xt)
                mask_sb = spool.tile([ec, CHUNK], _FP32, tag="mask")
                nc.scalar.dma_start(mask_sb[:], mask_d[:, c * CHUNK : (c + 1) * CHUNK])
                masks.append(mask_sb)
                if c == 0:
                    w1c_sb = wpool.tile([128, KH, EF_PAD], CDT)
                    nc.sync.dma_start(w1c_sb[:], w1c_d.rearrange("(o p) f -> p o f", p=128))
                    w2bd_sb = wpool.tile([128, KT3 * ec], CDT)
                    nc.sync.dma_start(w2bd_sb[:], w2bd_d[:])
                    bias_sb = wpool.tile([128, NB], _FP32)
                    nc.sync.dma_start(bias_sb[:], bias_d[:])
                    b2_sb = bias_sb[:ec, MH + KT3 : MH + KT3 + 1]
                    ones_sb = wpool.tile([ec, 1], SDT)
                    nc.vector.memset(ones_sb[:], 1.0)

            for c in range(N_CHUNKS):
                t0 = c * CHUNK
                xt = xts[c]
                mask_sb = masks[c]

                # ---- M1: hT = relu(W_shared.T @ xT + b) ----
                # chunk 0 runs k-outer so matmuls start as soon as the first
                # split DMA pieces land; later chunks are fully prefetched.
                hT = midpool.tile([128, MH, CHUNK], CDT, tag="hT")
                if c == 0:
                    phs = [ps_h.tile([128, CHUNK], _FP32, tag="ps_h", name=f"ph{m}") for m in range(MH)]
                    for k in range(KD):
                        for m in range(MH):
                            nc.tensor.matmul(
                                phs[m][:],
                                lhsT=wsh_sb[:, k, m * 128 : (m + 1) * 128],
                                rhs=xt[:, k, :],
                                start=(k == 0),
                                stop=(k == KD - 1),
                            )
                    for m in range(MH):
                        nc.scalar.activation(
                            hT[:, m, :], phs[m][:], relu, bias=bias_sb[:, m : m + 1]
                        )
                else:
                    for m in range(MH):
                        ph = ps_h.tile([128, CHUNK], _FP32, tag="ps_h")
                        for k in range(KD):
                            nc.tensor.matmul(
                                ph[:],
                                lhsT=wsh_sb[:, k, m * 128 : (m + 1) * 128],
                                rhs=xt[:, k, :],
                                start=(k == 0),
                                stop=(k == KD - 1),
                            )
                        nc.scalar.activation(
                            hT[:, m, :], ph[:], relu, bias=bias_sb[:, m : m + 1]
                        )

                # ---- M2: aT = relu(W1sel.T @ hT + b1) ----
                aT = midpool.tile([128, KT3, CHUNK], CDT, tag="aT")
                for m in range(KT3):
                    f0 = m * 128
                    pa = ps_a.tile([128, CHUNK], _FP32, tag="ps_a")
                    for k in range(KH):
                        nc.tensor.matmul(
                            pa[:],
                            lhsT=w1c_sb[:, k, f0 : f0 + 128],
                            rhs=hT[:, k, :],
                            start=(k == 0),
                            stop=(k == KH - 1),
                        )
                    nc.scalar.activation(
                        aT[:, m, :], pa[:], relu,
                        bias=bias_sb[:, MH + m : MH + m + 1],
                    )

                # ---- M3: c = W2sel.T @ aT  (block-diag W2) ----
                pc = ps_c.tile([ec, CHUNK], _FP32, tag="ps_c")
                for k in range(KT3):
                    nc.tensor.matmul(
                        pc[:],
                        lhsT=w2bd_sb[:, k * ec : (k + 1) * ec],
                        rhs=aT[:, k, :],
                        start=(k == 0),
                        stop=(k == KT3 - 1),
                    )

                # ---- select: out = ones.T @ ((c + b2) * mask) ----
                msel = spool.tile([ec, CHUNK], SDT, tag="msel")
                nc.vector.tensor_scalar_add(msel[:], pc[:], b2_sb)
                nc.vector.tensor_mul(msel[:], msel[:], mask_sb[:])
                po = ps_o.tile([1, CHUNK], _FP32, tag="ps_o")
                nc.tensor.matmul(po[:], lhsT=ones_sb[:], rhs=msel[:], start=True, stop=True)
                ot = spool.tile([1, CHUNK], _FP32, tag="ot")
                nc.vector.tensor_copy(ot[:], po[:])
                nc.gpsimd.dma_start(out_d[t0 : t0 + CHUNK].rearrange("(o t) -> o t", o=1), ot[:])

    nc.compile()
    return nc


def get_nc(ec):
    key = (COMPUTE_DT, ec)
    if key not in _cache:
        _cache[key] = _build_nc(ec)
    return _cache[key]


def prepare(inputs):
    """Host-side routing/sorting/sharding. Returns (ec, in_maps, perm)."""
    np_dt = _np_in_dtype()
    x = np.asarray(inputs["x"], dtype=np.float32)
    idx = np.asarray(inputs["idx"]).astype(np.int64).reshape(B)
    W_shared = np.asarray(inputs["W_shared"], dtype=np.float32)
    b_shared = np.asarray(inputs["b_shared"], dtype=np.float32).reshape(H)
    W1 = np.asarray(inputs["W1"], dtype=np.float32)
    b1 = np.asarray(inputs["b1"], dtype=np.float32).reshape(E, F)
    W2 = np.asarray(inputs["W2"], dtype=np.float32).reshape(E, F)
    b2 = np.asarray(inputs["b2"], dtype=np.float32).reshape(E)
    send_to = np.asarray(inputs["send_to"]).astype(np.int64)

    perm = np.argsort(idx, kind="stable")
    idx_s = idx[perm]
    routes_s = send_to[idx_s]                      # [B, K] sorted routes
    x_s = x[perm]                                  # [B, D]

    # per-core expert lists
    expert_lists = []
    for c in range(N_CORES):
        sl = slice(c * BL, (c + 1) * BL)
        expert_lists.append(np.unique(routes_s[sl]))
    ec = max(EC_MIN, max(len(el) for el in expert_lists))
    ec = min(ec, E)

    wsh = np.ascontiguousarray(W_shared).astype(np_dt)
    EF = ec * F
    KT3 = (EF + 127) // 128
    EF_PAD = KT3 * 128
    NB = MH + KT3 + 1

    in_maps = []
    for c in range(N_CORES):
        sl = slice(c * BL, (c + 1) * BL)
        el = expert_lists[c]
        # local slot tables (pad slots use sentinel -1: zero weights, no mask)
        slots = np.full(ec, -1, dtype=np.int64)
        slots[: len(el)] = el

        # mask[j, b] = (1/K) * count of slots[j] among routes of token b
        r = routes_s[sl]                            # [BL, K]
        mask = np.zeros((ec, BL), dtype=np.float32)
        for k in range(r.shape[1]):
            hit = slots[:, None] == r[None, :, k]   # [ec, BL]
            mask += hit.astype(np.float32) / r.shape[1]

        w1sel = np.zeros((H, EF_PAD), dtype=np.float32)
        b1sel = np.zeros(EF_PAD, dtype=np.float32)
        w2full = np.zeros((EF_PAD, ec), dtype=np.float32)
        for j, e in enumerate(slots):
            if e < 0:
                continue
            w1sel[:, j * F : (j + 1) * F] = W1[e]
            b1sel[j * F : (j + 1) * F] = b1[e]
            w2full[j * F : (j + 1) * F, j] = W2[e]
        w2bd = np.ascontiguousarray(
            w2full.reshape(KT3, 128, ec).transpose(1, 0, 2).reshape(128, KT3 * ec)
        ).astype(np_dt)

        biases = np.zeros((128, NB), dtype=np.float32)
        biases[:, :MH] = b_shared.reshape(MH, 128).T
        biases[:, MH : MH + KT3] = b1sel.reshape(KT3, 128).T
        biases[:ec, MH + KT3] = b2[np.maximum(slots, 0)] * (slots >= 0)

        xT = np.ascontiguousarray(
            x_s[sl].reshape(N_CHUNKS, CHUNK, D).transpose(0, 2, 1)
        ).astype(np_dt)

        in_maps.append(
            {
                "xT": xT,
                "mask": mask,
                "wsh": wsh,
                "w1c": w1sel.astype(np_dt),
                "w2bd": w2bd,
                "biases": biases,
            }
        )
    return ec, in_maps, perm


def kernel(**inputs) -> np.ndarray:
    ec, in_maps, perm = prepare(inputs)
    nc = get_nc(ec)
    res = run_bass_kernel_spmd(nc, in_maps, list(range(N_CORES)))
    out_sorted = np.concatenate([res.results[c]["out"] for c in range(N_CORES)])
    out = np.empty(B, dtype=np.float32)
    out[perm] = out_sorted
    return out.reshape(B, 1)
